# revision 32
# baseline (speedup 1.0000x reference)
"""Trainium2 Bass kernel for nn_Linear_18494129177115 (moe_routing).

Math (reference, fp32):
  base   = x @ W^T                                  [B,T,O]
  logits = x @ Wr^T + lang_bias                     [B,T,E]
  gates  = scatter(softmax(top2(logits)))           [B,T,E]
  h      = x @ A_e^T  (all experts)                 [B,T,E,R]
  out    = base + SCALING * sum_e gates_e * h_e @ B_e^T

Key design points:
- With A_cat = concat_e(A_e) [E*R, D] and B_cat[e*R+r, o] = B[e, o, r],
  the gated LoRA collapses to
    out = x @ W^T + (gates_expanded * (x @ A_cat^T)) @ (SCALING * B_cat),
  two thin matmuls fused into the base GEMM's PSUM accumulation.
- The tolerance gate is 2e-2 absmax-relative; a SINGLE bf16 pass of the
  heavy GEMMs lands at ~2.3e-3 (measured vs fp32 reference on the
  grading seed), so the base GEMM, h, and the B_cat matmul all run one
  bf16 pass (3x fewer PE cycles than the hi/lo 3-pass split).
- The router alone keeps a 3-pass bf16 hi/lo split (xh@Wrh + xl@Wrh +
  xh@Wrl): top-2 selection flips from single-pass logit noise cost up
  to ~1e-2 absmax; 3-pass logits are ~1e-5 accurate so selection
  matches fp32. Wr is tiny so this costs ~30us of PE.
- DMA: all host-side layouts are partition-major so every transfer is
  a contiguous >=1MB block; W streams in 1MB chunks over the two HWDGE
  rings, x-lo streams on the SWDGE ring, outputs pair-batched to 512KB.

Sharding: data-parallel over tokens, 1024 tokens/core on 8 cores; all
weights replicated; no collectives. Each core's tokens lie in a single
batch row, so the language bias is a per-core constant [E,1] column.
"""

import numpy as np

LANG_BIAS = 5.0
SCALING = 32.0 / 16.0
B_SZ, T_SZ, D_SZ, O_SZ, E_SZ, R_SZ = 4, 2048, 4096, 4096, 8, 16
NCORES = 8
TPC = (B_SZ * T_SZ) // NCORES      # 1024 tokens per core
NT = TPC // 128                    # 8 token tiles per core
NK = D_SZ // 128                   # 32 contraction chunks
NO = O_SZ // 512                   # 8 output tiles of 512
ER = E_SZ * R_SZ                   # 128 (expert, rank) pairs
NEG_BIG = -(2.0 ** 100)

_CACHE: dict = {}
LAST_RESULT = None


def _build_bass(loop_n=None):
    import concourse.bacc as bacc
    import concourse.mybir as mybir
    from concourse import tile
    from concourse.masks import make_identity

    f32 = mybir.dt.float32
    bf16 = mybir.dt.bfloat16
    AX = mybir.AxisListType.X
    OP = mybir.AluOpType
    ACT = mybir.ActivationFunctionType

    nc = bacc.Bacc(None, target_bir_lowering=False, debug=False)

    # x hi, partition-major: [128, kc, t]
    xh_d = nc.dram_tensor("xh", [128, NK, TPC], bf16, kind="ExternalInput")
    # x lo, grouped for streaming: [g4, 128, 4, t]
    xl_d = nc.dram_tensor("xl", [NK // 4, 128, 4, TPC], bf16, kind="ExternalInput")
    # W^T stream: per (ot, g8) a [128, 8kc, 512] block
    wt_d = nc.dram_tensor("wt", [NO, NK // 8, 128, 8, 512], bf16, kind="ExternalInput")
    # A_cat^T resident: [128, kc, ER]
    acat_d = nc.dram_tensor("acat", [128, NK, ER], bf16, kind="ExternalInput")
    # Wr^T hi/lo: [2, 128, kc, E]
    wrt_d = nc.dram_tensor("wrt", [2, 128, NK, E_SZ], bf16, kind="ExternalInput")
    # Wr^T stacked [wrh | wrl]: [128, kc, 2E]
    wr2_d = nc.dram_tensor("wr2", [128, NK, 2 * E_SZ], bf16, kind="ExternalInput")
    # SCALING * B_cat, bf16: [ER, O]
    bcat_d = nc.dram_tensor("bcat", [ER, O_SZ], bf16, kind="ExternalInput")
    # language bias column [E, 1]
    bias_d = nc.dram_tensor("biasr", [E_SZ, 1], f32, kind="ExternalInput")
    # expert -> (expert, rank) one-hot expansion [E, ER]
    sel_d = nc.dram_tensor("sel", [E_SZ, ER], f32, kind="ExternalInput")
    out_d = nc.dram_tensor("out", [NO, NT // 2, 128, 2, 512], f32, kind="ExternalOutput")

    with tile.TileContext(nc) as tc:
        with (
            tc.tile_pool(name="const", bufs=1) as cpool,
            tc.tile_pool(name="wstream", bufs=4) as wpool,
            tc.tile_pool(name="xlstream", bufs=6) as xlpool,
            tc.tile_pool(name="ostage", bufs=3) as opool,
            tc.tile_pool(name="gate", bufs=2) as gpool,
            tc.tile_pool(name="psum", bufs=8, space="PSUM") as psum,
        ):

          def body(_iv=None):
            # ---- resident inputs ----
            # x-hi group sizes: small first chunks so the first matmuls
            # start ~2.5us in instead of waiting on a full 1MB chunk
            XH_SZ = [2, 2, 4, 4, 4, 4, 4, 4, 4]
            XH_OFF = [sum(XH_SZ[:i]) for i in range(len(XH_SZ) + 1)]
            xh_g = [
                cpool.tile([128, s, TPC], bf16, name=f"xh_g{g}")
                for g, s in enumerate(XH_SZ)
            ]
            AC_SZ = [4, 4, 8, 8, 8]
            AC_OFF = [sum(AC_SZ[:i]) for i in range(len(AC_SZ) + 1)]
            acat_c = [
                cpool.tile([128, s, ER], bf16, name=f"acat_c{c}")
                for c, s in enumerate(AC_SZ)
            ]
            wrh_sb = cpool.tile([128, NK, E_SZ], bf16, name="wrh_sb")
            wr2_sb = cpool.tile([128, NK, 2 * E_SZ], bf16, name="wr2_sb")
            bch_sb = cpool.tile([ER, O_SZ], bf16, name="bch_sb")
            bias_sb = cpool.tile([E_SZ, 1], f32, name="bias_sb")
            sel_sb = cpool.tile([E_SZ, ER], f32, name="sel_sb")
            ident_sb = cpool.tile([128, 128], f32, name="ident_sb")
            hT_sb = cpool.tile([128, TPC], f32, name="hT_sb")
            ghh_sb = cpool.tile([128, NT, 128], bf16, name="ghh_sb")
            lgT_sb = cpool.tile([E_SZ, TPC], f32, name="lgT_sb")

            import bisect

            def xh(kc):
                g = bisect.bisect_right(XH_OFF, kc) - 1
                return xh_g[g][:, kc - XH_OFF[g], :]

            def ac(kc):
                c = bisect.bisect_right(AC_OFF, kc) - 1
                return acat_c[c][:, kc - AC_OFF[c], :]

            # x-hi chunks alternate the two HWDGE rings in demand order;
            # A_cat chunks + x-lo stream on the SWDGE ring; small weights
            # lead their rings so the first matmuls start early
            nc.sync.dma_start(wrh_sb[:], wrt_d[0])
            nc.scalar.dma_start(wr2_sb[:], wr2_d[:])
            for c in range(2):
                nc.gpsimd.dma_start(
                    acat_c[c][:], acat_d[:, AC_OFF[c] : AC_OFF[c + 1], :]
                )
            for g in range(len(XH_SZ)):
                eng = nc.sync if g % 2 == 0 else nc.scalar
                eng.dma_start(xh_g[g][:], xh_d[:, XH_OFF[g] : XH_OFF[g + 1], :])
                if g == 2:
                    nc.scalar.dma_start(bias_sb[:], bias_d[:])
                    nc.sync.dma_start(sel_sb[:], sel_d[:])
            for c in range(2, len(AC_SZ)):
                nc.gpsimd.dma_start(
                    acat_c[c][:], acat_d[:, AC_OFF[c] : AC_OFF[c + 1], :]
                )
            xl_t = []
            for kg in range(NK // 4):
                t = xlpool.tile([128, 4, TPC], bf16, tag="xl", name=f"xl{kg}")
                nc.gpsimd.dma_start(t[:], xl_d[kg])
                xl_t.append(t)
            nc.scalar.dma_start(bch_sb[:], bcat_d[:])
            make_identity(nc, ident_sb[:])

            # ---- phase 1: router hi-passes + h, one stream over x-hi ----
            plT = [
                psum.tile([E_SZ, 512], f32, tag="bank", name=f"plT{t}")
                for t in range(TPC // 512)
            ]
            ph = [
                psum.tile([128, 512], f32, tag="bank", name=f"ph{t}")
                for t in range(TPC // 512)
            ]
            for kc in range(NK):
                first = kc == 0
                last = kc == NK - 1
                ack = ac(kc)
                for tb in range(TPC // 512):
                    sl = slice(tb * 512, (tb + 1) * 512)
                    nc.tensor.matmul(
                        ph[tb][:], ack, xh(kc)[:, sl],
                        start=first, stop=last,
                    )
            # router lo-pass (xl arrived during the loop above); sole
            # writer of plT so kc==0 opens the accumulation group
            for kc in range(NK):
                first = kc == 0
                last = kc == NK - 1
                for tb in range(TPC // 512):
                    sl = slice(tb * 512, (tb + 1) * 512)
                    nc.tensor.matmul(
                        plT[tb][:], wrh_sb[:, kc, :],
                        xl_t[kc // 4][:, kc % 4, sl],
                        start=first, stop=last,
                    )
            # router hi-pass, token-major: stationary x tile, stacked
            # [wrh | wrl] moving; tt-outer so each token tile's PSUM
            # accumulation group opens and closes before the next (a bank
            # allows only one pending group per zero region)
            plgh = psum.tile([128, NT, 2 * E_SZ], f32, tag="bank", name="plgh")
            for tt in range(NT):
                ts = slice(tt * 128, (tt + 1) * 128)
                for kc in range(NK):
                    nc.tensor.matmul(
                        plgh[:, tt, :], xh(kc)[:, ts], wr2_sb[:, kc, :],
                        start=(kc == 0), stop=(kc == NK - 1),
                    )
            for tb in range(TPC // 512):
                sl = slice(tb * 512, (tb + 1) * 512)
                # fold the language bias into the PSUM drain (bias is a
                # per-partition [E,1] column in this transposed layout)
                nc.vector.tensor_scalar(
                    lgT_sb[:, sl], plT[tb][:], bias_sb[:], None, op0=OP.add
                )
                nc.vector.tensor_copy(hT_sb[:, sl], ph[tb][:])
            lgh2_sb = cpool.tile([128, NT, 2 * E_SZ], f32, name="lgh2_sb")
            nc.vector.tensor_copy(lgh2_sb[:], plgh[:])
            lgh_sb = cpool.tile([128, NT, E_SZ], f32, name="lgh_sb")
            nc.vector.tensor_tensor(
                lgh_sb[:], lgh2_sb[:, :, :E_SZ], lgh2_sb[:, :, E_SZ:], op=OP.add
            )

            # ---- gate softmax chain (emitted inside ot=0's first half
            # below so the DVE work hides under base-GEMM matmuls)
            gates_t = [None] * NT

            def gate_chain(tt):
                ts = slice(tt * 128, (tt + 1) * 128)
                plo = psum.tile([128, E_SZ], f32, tag="bank", name=f"plo{tt}")
                nc.tensor.transpose(plo[:], lgT_sb[:, ts], ident_sb[:E_SZ, :E_SZ])
                plg = gpool.tile([128, E_SZ], f32, name="logit")
                nc.vector.tensor_tensor(
                    plg[:], lgh_sb[:, tt, :], plo[:], op=OP.add
                )
                m1 = gpool.tile([128, 1], f32, name="m1")
                nc.vector.reduce_max(m1[:], plg[:], axis=AX)
                mask1 = gpool.tile([128, E_SZ], f32, name="mask1")
                nc.vector.tensor_scalar(
                    mask1[:], plg[:], m1[:], None, op0=OP.is_equal
                )
                l2 = gpool.tile([128, E_SZ], f32, name="l2")
                nc.vector.tensor_scalar(l2[:], mask1[:], NEG_BIG, None, op0=OP.mult)
                nc.vector.tensor_tensor(l2[:], l2[:], plg[:], op=OP.add)
                m2 = gpool.tile([128, 1], f32, name="m2")
                nc.vector.reduce_max(m2[:], l2[:], axis=AX)
                mask2 = gpool.tile([128, E_SZ], f32, name="mask2")
                nc.vector.tensor_scalar(
                    mask2[:], l2[:], m2[:], None, op0=OP.is_equal
                )
                w1 = gpool.tile([128, 1], f32, name="w1")
                nc.scalar.activation(
                    w1[:], m2[:], ACT.Sigmoid, bias=m1[:], scale=-1.0
                )
                w2 = gpool.tile([128, 1], f32, name="w2")
                nc.vector.tensor_scalar(
                    w2[:], w1[:], -1.0, 1.0, op0=OP.mult, op1=OP.add
                )
                g1 = gpool.tile([128, E_SZ], f32, name="g1")
                nc.vector.tensor_scalar(g1[:], mask1[:], w1[:], None, op0=OP.mult)
                gates = gpool.tile([128, E_SZ], f32, name=f"gates{tt}")
                nc.vector.tensor_scalar(
                    gates[:], mask2[:], w2[:], None, op0=OP.mult
                )
                nc.vector.tensor_tensor(gates[:], gates[:], g1[:], op=OP.add)
                gates_t[tt] = gates

            obs = {}

            def tail_tt(po, ot, tt, tt0):
                # lora matmul closes the accumulation; copy to the staging
                # pair tile and DMA out when the pair completes
                osl = slice(ot * 512, (ot + 1) * 512)
                j = tt % 2
                if j == 0:
                    obs[tt // 2] = opool.tile([128, 2, 512], f32, name="ob")
                ob = obs[tt // 2]
                nc.tensor.matmul(
                    po[tt - tt0][:], ghh_sb[:, tt, :], bch_sb[:, osl],
                    start=False, stop=True,
                )
                nc.vector.tensor_copy(ob[:, j, :], po[tt - tt0][:])
                if j == 1:
                    if ot == NO - 1:
                        oeng = (nc.gpsimd, nc.sync, nc.scalar)[(tt // 2) % 3]
                    else:
                        oeng = nc.gpsimd
                    oeng.dma_start(out_d[ot, tt // 2], ob[:])

            def base_mms(po, w_t, g, tts, tt0, tail_ot=None):
                # tail_ot set: fuse each token tile's lora tail right after
                # its last base matmul so the PSUM bank frees ~2us before
                # the next ot pass needs it
                for k8 in range(8):
                    kc = g * 8 + k8
                    for tt in tts:
                        nc.tensor.matmul(
                            po[tt - tt0][:],
                            xh(kc)[:, tt * 128 : (tt + 1) * 128],
                            w_t[:, k8, :],
                            start=(kc == 0),
                            stop=False,
                        )
                        if tail_ot is not None and kc == NK - 1:
                            tail_tt(po, tail_ot, tt, tt0)

            def tails(po, ot, tts, tt0):
                for tt in tts:
                    tail_tt(po, ot, tt, tt0)

            # ---- phase 2, ot=0: two half-passes of 4 token tiles each so
            # 4 PSUM banks stay free for the gate chain; the 4 W chunks are
            # held resident and reused by the second half
            po_a = [
                psum.tile([128, 512], f32, tag="bank", name=f"poa{i}")
                for i in range(4)
            ]
            for g in range(4):
                w_t = wpool.tile([128, 8, 512], bf16, name="w_t")
                eng = nc.sync if g % 2 == 0 else nc.scalar
                eng.dma_start(w_t[:], wt_d[0, g])
                base_mms(po_a, w_t, g, range(4), 0)
                gate_chain(2 * g)
                gate_chain(2 * g + 1)
            # gate finalize: expand gates to (e,r) rows, ghh = gates_exp * h
            for tt in range(NT):
                ts = slice(tt * 128, (tt + 1) * 128)
                ptr = psum.tile([E_SZ, 128], f32, tag="bank", name=f"ptr{tt}")
                nc.tensor.transpose(ptr[:], gates_t[tt][:], ident_sb[:])
                gT = gpool.tile([E_SZ, 128], f32, name="gT")
                nc.vector.tensor_copy(gT[:], ptr[:])
                pge = psum.tile([128, 128], f32, tag="bank", name=f"pge{tt}")
                nc.tensor.matmul(pge[:], sel_sb[:], gT[:], start=True, stop=True)
                nc.vector.tensor_tensor(
                    ghh_sb[:, tt, :], pge[:], hT_sb[:, ts], op=OP.mult
                )
            tails(po_a, 0, [0, 1, 2, 3], 0)
            po_b = [
                psum.tile([128, 512], f32, tag="bank", name=f"pob{i}")
                for i in range(4)
            ]
            for g in range(4):
                w_t = wpool.tile([128, 8, 512], bf16, name="w_t")
                eng = nc.scalar if g % 2 == 0 else nc.sync
                eng.dma_start(w_t[:], wt_d[0, g])
                base_mms(po_b, w_t, g, range(4, 8), 4, tail_ot=0 if g == 3 else None)

            # ---- phase 2, ot=1..7: streaming full-width passes ----
            for ot in range(1, NO):
                po = [
                    psum.tile([128, 512], f32, tag="bank", name=f"po{ot}_{i}")
                    for i in range(NT)
                ]
                for g in range(NK // 8):
                    w_t = wpool.tile([128, 8, 512], bf16, name="w_t")
                    eng = nc.sync if (ot * 4 + g) % 2 == 0 else nc.scalar
                    eng.dma_start(w_t[:], wt_d[ot, g])
                    base_mms(po, w_t, g, range(NT), 0, tail_ot=ot if g == 3 else None)

          if loop_n is None:
              body()
          else:
              with tc.For_i(0, loop_n, 1) as iv:
                  body(iv)

    nc.compile()
    return nc


def _split_bf16(a):
    import ml_dtypes

    hi = a.astype(ml_dtypes.bfloat16)
    lo = (a - hi.astype(np.float32)).astype(ml_dtypes.bfloat16)
    return hi, lo


def _host_prep(x, language_ids, W, Wr, A, B):
    x = np.asarray(x, dtype=np.float32)
    W = np.asarray(W, dtype=np.float32)
    Wr = np.asarray(Wr, dtype=np.float32)
    A = np.asarray(A, dtype=np.float32)
    B = np.asarray(B, dtype=np.float32)
    lang = np.asarray(language_ids).astype(np.int64)

    xf = np.ascontiguousarray(x.reshape(B_SZ * T_SZ, D_SZ))

    # W^T [D,O] bf16 hi: [ot, g8, p, k8, n]
    wtT = W.T.reshape(NK, 128, NO, 512)                   # [kc, p, ot, n]
    wh, _ = _split_bf16(wtT)
    wt = np.ascontiguousarray(
        wh.reshape(NK // 8, 8, 128, NO, 512).transpose(3, 0, 2, 1, 4)
    )

    acat_t = np.ascontiguousarray(A.reshape(ER, D_SZ).T).reshape(NK, 128, ER)
    ah, _ = _split_bf16(acat_t)
    acat = np.ascontiguousarray(ah.transpose(1, 0, 2))    # [p, kc, ER]

    wrtT = np.ascontiguousarray(Wr.T).reshape(NK, 128, E_SZ)
    wrh, wrl = _split_bf16(wrtT)
    wrt = np.ascontiguousarray(
        np.stack([wrh, wrl], axis=0).transpose(0, 2, 1, 3)  # [2, p, kc, E]
    )
    wr2 = np.ascontiguousarray(
        np.concatenate([wrh, wrl], axis=2).transpose(1, 0, 2)  # [p, kc, 2E]
    )

    bcat32 = (SCALING * B.transpose(0, 2, 1)).reshape(ER, O_SZ)
    bh, _ = _split_bf16(bcat32)
    bcat = np.ascontiguousarray(bh)

    sel = np.zeros((E_SZ, ER), dtype=np.float32)
    sel[np.arange(ER) // R_SZ, np.arange(ER)] = 1.0

    in_maps = []
    for c in range(NCORES):
        shard = xf[c * TPC : (c + 1) * TPC]
        xr = np.ascontiguousarray(shard.T).reshape(NK, 128, TPC)
        xhh, xll = _split_bf16(xr)
        xh = np.ascontiguousarray(xhh.transpose(1, 0, 2))             # [p, kc, t]
        xl = np.ascontiguousarray(
            xll.reshape(NK // 4, 4, 128, TPC).transpose(0, 2, 1, 3)   # [g, p, 4, t]
        )
        b = int(lang[(c * TPC) // T_SZ])
        brow = np.zeros((E_SZ, 1), dtype=np.float32)
        if b >= 0:
            brow[b, 0] = LANG_BIAS
        in_maps.append(
            {
                "xh": xh,
                "xl": xl,
                "wt": wt,
                "acat": acat,
                "wrt": wrt,
                "wr2": wr2,
                "bcat": bcat,
                "biasr": brow,
                "sel": sel,
            }
        )
    return in_maps


def kernel(x, language_ids, W, Wr, A, B):
    global LAST_RESULT
    from concourse.bass_utils import run_bass_kernel_spmd

    if "nc" not in _CACHE:
        _CACHE["nc"] = _build_bass()
    nc = _CACHE["nc"]

    in_maps = _host_prep(x, language_ids, W, Wr, A, B)
    res = run_bass_kernel_spmd(nc, in_maps, core_ids=list(range(NCORES)))
    LAST_RESULT = res
    outs = [
        r["out"].transpose(1, 3, 2, 0, 4).reshape(TPC, O_SZ) for r in res.results
    ]
    return np.concatenate(outs, axis=0).reshape(B_SZ, T_SZ, O_SZ)
